# revision 22
# baseline (speedup 1.0000x reference)
"""DecoderRNN (LSTM image-caption decoder) on 8 TRN2 NeuronCores.

Sharding: data-parallel over batch B=128 -> 16 rows per core. No collectives.
Host side does the data-dependent index work (length sort, embedding gather)
and packs operands into PE-friendly layouts; the device runs:
  1. X-proj: x_t @ W_ih^T + bias for all 20 steps as one batched matmul sweep
  2. 19 sequential LSTM-cell steps (h @ W_hh^T, gate nonlinearities)
  3. FC to vocab (10000) for all 19*16 rows, fused (psum+bias)*mask epilogue
All matmuls in bf16 with f32 PSUM accumulation.

Perf notes (trace-driven):
  - each LSTM gate group accumulates in its own PSUM bank so the scalar
    engine can start on a gate while later gates are still in matmul
  - the x-projection is injected into each gate's PSUM accumulation via an
    identity matmul, so gate pre-activations are complete in PSUM and the
    scalar engine reads them directly (no vector-engine add pass)
  - bank-clear semantics: only the first matmul of a gate group uses
    start=True (clears the whole bank's has_written bits); every other
    matmul in the group overwrites-where-unset / accumulates-where-set
"""

import numpy as np
import ml_dtypes

B, S, E, H, V = 128, 20, 512, 512, 10000
T = S - 1            # 19 decode steps
NCORES = 8
BC = B // NCORES     # 16 batch rows per core
NSTEP = S            # 20 LSTM cell evaluations (features step + 19 caption steps)
RX = NSTEP * BC      # 320 x-proj rows per core
R = T * BC           # 304 fc rows per core
KC = E // 128        # 4 contraction chunks (E == H == 512)
MT = 4 * H // 128    # 16 gate m-tiles
VT = (V + 127) // 128  # 79 vocab tiles
R1 = 8 * BC            # fc rows computed inside the recurrence (steps 0..7)
R2 = R - R1            # fc rows computed after the last step
VP = VT * 128          # vocab padded to a whole number of tiles

PROFILE = False      # set True (from test.py) to capture NTFF trace + exec time
LAST_RESULT = None   # BassKernelResults of the last run (for test.py)

_COMPILED = None


def _build():
    import concourse.mybir as mybir
    import concourse.tile as tile
    from concourse import bacc
    from concourse.masks import make_identity
    from contextlib import ExitStack

    f32 = mybir.dt.float32
    bf16 = mybir.dt.bfloat16
    AF = mybir.ActivationFunctionType
    OP = mybir.AluOpType

    nc = bacc.Bacc(None)

    # all DRAM tensors are laid out [128 partitions, ...contiguous...] so each
    # partition's payload is a single contiguous run (DMA descriptor-count,
    # not bandwidth, was the staging bottleneck with row-major layouts)
    xk = nc.declare_dram_parameter("xk", [128, KC, RX], bf16, isOutput=False)
    wih = nc.declare_dram_parameter("wih", [128, MT, KC, 128], bf16, isOutput=False)
    whh = nc.declare_dram_parameter("whh", [128, KC, 4 * H], bf16, isOutput=False)
    bias = nc.declare_dram_parameter("bias", [128, MT], f32, isOutput=False)
    fcw = nc.declare_dram_parameter("fcw", [128, KC, VP], bf16, isOutput=False)
    fcb = nc.declare_dram_parameter("fcb", [128, VT], f32, isOutput=False)
    mask = nc.declare_dram_parameter("mask", [128, R], f32, isOutput=False)
    preds1 = nc.declare_dram_parameter("preds1", [128, VT * R1], bf16, isOutput=True)
    preds2 = nc.declare_dram_parameter("preds2", [128, VT * R2], bf16, isOutput=True)

    xk_r, wih_r, whh_r, fcw_r = xk, wih, whh, fcw

    with tile.TileContext(nc) as tc, ExitStack() as ctx:
        const = ctx.enter_context(tc.tile_pool(name="const", bufs=1))
        gates = ctx.enter_context(tc.tile_pool(name="gates", bufs=2))
        fcout = ctx.enter_context(tc.tile_pool(name="fcout", bufs=3))
        # one bank per slot; x-proj + fc share the 4 rotating "big" slots,
        # the four gate groups get a dedicated bank each (4 + 4 <= 8 banks)
        ps_big = ctx.enter_context(tc.tile_pool(name="ps_big", bufs=4, space="PSUM"))
        ps_gate = ctx.enter_context(tc.tile_pool(name="ps_gate", bufs=1, space="PSUM"))

        ident = const.tile([128, 128], bf16)

        # PE warmup: dummy matmuls on an *uninitialized* scratch tile (no
        # dependencies -> issues immediately at kernel start) keep the PE busy
        # while the input DMAs stream, so the HAM clock gate reaches 2.4 GHz
        # before the first real matmul and stays there
        junk = const.tile([128, 128], bf16)
        nc.gpsimd.memset(junk[:], 0.0)
        ps_warm = ps_gate.tile([128, 4, BC], f32, tag="pg_g")
        for _ in range(160):
            nc.tensor.matmul(ps_warm[:, :, :], junk[:],
                             junk[:, :4 * BC].rearrange("p (a b) -> p a b", b=BC),
                             start=True, stop=True, skip_group_check=True)
        make_identity(nc, ident[:])

        # ---- stage weights/inputs into SBUF ----
        # one dma_start per tensor: DIRECT2D descriptor generation costs
        # ~0.6us *serially* on the issuing sequencer, so few big transfers
        # beat many small ones; fcw goes on the gpsimd sequencer so the sync
        # sequencer is free for the X-proj-critical loads
        # ---- stage weights/inputs into SBUF ----
        # many mid-size transfers spread across the 16 DMA queues; the
        # X-proj-critical loads (xk, wih) are issued first
        xk_sb = const.tile([128, KC, RX], bf16)
        for k in range(KC):
            nc.sync.dma_start(xk_sb[:, k, :], xk_r[:, k, :])
        # wih is mt-major so each quarter unlocks 4 complete X-proj groups
        wih_sb = const.tile([128, MT, KC, 128], bf16)
        for mq in range(4):
            nc.sync.dma_start(wih_sb[:, mq * 4:(mq + 1) * 4], wih_r[:, mq * 4:(mq + 1) * 4])
        bias_sb = const.tile([128, MT], f32)
        nc.sync.dma_start(bias_sb[:], bias[:])
        whh_sb = const.tile([128, KC, 4 * H], bf16)
        for k in range(KC):
            nc.sync.dma_start(whh_sb[:, k, :], whh_r[:, k, :])
        mask_sb = const.tile([128, R], f32)
        nc.sync.dma_start(mask_sb[:], mask[:])
        fcb_sb = const.tile([128, VT], f32)
        nc.sync.dma_start(fcb_sb[:], fcb[:])
        fcw_sb = const.tile([128, KC, VP], bf16)
        vtq = [0, 20 * 128, 40 * 128, 60 * 128, VP]
        for q in range(4):
            for k in range(KC):
                nc.sync.dma_start(fcw_sb[:, k, vtq[q]:vtq[q + 1]],
                                  fcw_r[:, k, vtq[q]:vtq[q + 1]])


        xp_sb = const.tile([128, MT, RX], bf16)  # x-projections + bias, all steps
        h_all = const.tile([128, KC, RX], bf16)  # h_t for every step (k-major)
        c_sb = const.tile([128, KC, BC], f32)    # cell state

        # ---- X-proj: xp[:, mt, r] = sum_k W_ihT[k, mt*128:+128].T @ x + bias ----
        for mt in range(MT):
            ps = ps_big.tile([128, RX], f32, tag="big")
            for k in range(KC):
                nc.tensor.matmul(
                    ps[:],
                    wih_sb[:, mt, k, :],
                    xk_sb[:, k, :],
                    start=(k == 0),
                    stop=(k == KC - 1),
                )
            nc.scalar.activation(
                xp_sb[:, mt, :], ps[:], AF.Identity,
                bias=bias_sb[:, mt:mt + 1], scale=1.0,
            )

        # gate m-tile bases: torch LSTMCell gate order is i, f, g, o
        I0, F0, G0, O0 = 0, 4, 8, 12

        def xp_t(g0, s):
            return xp_sb[:, g0:g0 + 4, s * BC:(s + 1) * BC]


        def fc_groups(g_list, r_lo, r_nw, preds_t, stg_tag, gsz=4, split_epi=False):
            # r_lo/r_nw: fc row window; h_all col = 16 + r
            for gi, g0 in enumerate(g_list):
                gn = min(gsz, VT - g0)
                stage = fcout.tile([128, gsz, r_nw], bf16, tag=stg_tag)
                for gj in range(gn):
                    vt = g0 + gj
                    ps = ps_big.tile([128, R], f32, tag="big")
                    for k in range(KC):
                        nc.tensor.matmul(
                            ps[:, :r_nw],
                            fcw_sb[:, k, vt * 128:(vt + 1) * 128],
                            h_all[:, k, BC + r_lo:BC + r_lo + r_nw],
                            start=(k == 0),
                            stop=(k == KC - 1),
                        )
                    if split_epi and vt % 2 == 1:
                        tmp = fcout.tile([128, r_nw], f32, tag="epi_tmp")
                        nc.scalar.activation(
                            tmp[:], ps[:, :r_nw], AF.Identity,
                            bias=fcb_sb[:, vt:vt + 1], scale=1.0,
                        )
                        nc.gpsimd.tensor_mul(
                            stage[:, gj, :], tmp[:],
                            mask_sb[:, r_lo:r_lo + r_nw],
                        )
                    else:
                        nc.vector.scalar_tensor_tensor(
                            stage[:, gj, :], ps[:, :r_nw], fcb_sb[:, vt:vt + 1],
                            mask_sb[:, r_lo:r_lo + r_nw], OP.add, OP.mult,
                        )
                eng = nc.sync if (g0 // gsz) % 2 == 0 else nc.gpsimd
                eng.dma_start(
                    preds_t[:, g0 * r_nw:(g0 + gn) * r_nw].rearrange(
                        "p (g r) -> p g r", r=r_nw),
                    stage[:, :gn, :],
                )

        # ---- step 0: h,c from features only (h_prev = 0, c_prev = 0) ----
        tg = gates.tile([128, 4, BC], f32)
        nc.scalar.activation(tg[:], xp_t(G0, 0), AF.Tanh)
        si = gates.tile([128, 4, BC], f32)
        nc.scalar.activation(si[:], xp_t(I0, 0), AF.Sigmoid)
        so = gates.tile([128, 4, BC], f32)
        nc.scalar.activation(so[:], xp_t(O0, 0), AF.Sigmoid)
        nc.vector.tensor_tensor(c_sb[:], si[:], tg[:], OP.mult)
        tc_ = gates.tile([128, 4, BC], f32)
        nc.scalar.activation(tc_[:], c_sb[:], AF.Tanh)
        nc.vector.tensor_tensor(h_all[:, :, 0:BC], so[:], tc_[:], OP.mult)

        # chunk-1 fc group schedule: 20 groups of 4 vocab tiles spread
        # over steps 9..19
        g_all = list(range(0, VT, 4))   # 20 groups of 4 vocab tiles
        chunk1_sched = {}
        gi = 0
        for s9 in range(9, NSTEP):
            take = 2 if gi + 2 <= len(g_all) else len(g_all) - gi
            if s9 == NSTEP - 1:
                take = len(g_all) - gi
            chunk1_sched[s9] = g_all[gi:gi + take]
            gi += take

        # ---- steps 1..19: full LSTM cell ----
        # gate group issue order g, f, i, o: the c-update chain (needs g, f, i)
        # runs while the o matmuls stream; the o tail is short
        for s in range(1, NSTEP):
            h_prev = h_all[:, :, (s - 1) * BC:s * BC]

            def gate_mms(g0, tag):
                ps = ps_gate.tile([128, 4, BC], f32, tag=tag)
                first = True
                for j in range(4):
                    for k in range(KC):
                        nc.tensor.matmul(
                            ps[:, j, :],
                            whh_sb[:, k, (g0 + j) * 128:(g0 + j + 1) * 128],
                            h_prev[:, k, :],
                            start=first,
                            stop=False,
                            skip_group_check=True,
                        )
                        first = False
                # inject x-proj (+bias) via identity matmul: completes the
                # gate pre-activation entirely inside PSUM
                nc.tensor.matmul(
                    ps[:, :, :], ident[:], xp_t(g0, s),
                    start=False, stop=True, skip_group_check=True,
                )
                return ps

        # g
            ps_g = gate_mms(G0, "pg_g")
            tg = gates.tile([128, 4, BC], f32)
            nc.scalar.activation(tg[:], ps_g[:, :, :], AF.Tanh)
        # f
            ps_f = gate_mms(F0, "pg_f")
            sf = gates.tile([128, 4, BC], f32)
            nc.scalar.activation(sf[:], ps_f[:, :, :], AF.Sigmoid)
            c1 = gates.tile([128, 4, BC], f32)
            nc.vector.tensor_tensor(c1[:], sf[:], c_sb[:], OP.mult)
        # i
            ps_i = gate_mms(I0, "pg_i")
            si = gates.tile([128, 4, BC], f32)
            nc.scalar.activation(si[:], ps_i[:, :, :], AF.Sigmoid)
            t2 = gates.tile([128, 4, BC], f32)
            nc.vector.tensor_tensor(t2[:], si[:], tg[:], OP.mult)
            nc.vector.tensor_tensor(c_sb[:], c1[:], t2[:], OP.add)
        # o
            ps_o = gate_mms(O0, "pg_o")
            so = gates.tile([128, 4, BC], f32)
            nc.scalar.activation(so[:], ps_o[:, :, :], AF.Sigmoid)
            tc_ = gates.tile([128, 4, BC], f32)
            nc.scalar.activation(tc_[:], c_sb[:], AF.Tanh)
            nc.vector.tensor_tensor(
                h_all[:, :, s * BC:(s + 1) * BC], so[:], tc_[:], OP.mult
            )
            # fc chunk 1 (rows for t=0..7) rides the PE/DVE gaps of the
            # gate chain once those h rows exist
            if s >= 9:
                fc_groups(chunk1_sched.get(s, []), 0, R1, preds1, "st1")
            else:
                # keep-warm filler: the gate chain leaves the PE idle ~2.4us
                # per step, enough for the HAM clock gate to re-throttle to
                # 1.2 GHz; ~1us of dummy matmuls keeps it at 2.4 GHz
                ps_d = ps_big.tile([128, RX], f32, tag="big")
                for k in range(KC):
                    for _ in range(2):
                        nc.tensor.matmul(ps_d[:], junk[:], xk_sb[:, k, :],
                                         start=True, stop=True,
                                         skip_group_check=True)

        # ---- FC tail: rows t=8..18 for every vocab tile ----
        fc_groups(list(range(0, VT, 4)), R1, R2, preds2, "st2", split_epi=True)

    nc.compile()
    return nc


def _get_compiled():
    global _COMPILED
    if _COMPILED is None:
        _COMPILED = _build()
    return _COMPILED


def kernel(images, captions, length, emb, W_ih, W_hh, b_ih, b_hh, fc_w, fc_b):
    global LAST_RESULT
    from concourse.bass_utils import run_bass_kernel_spmd

    images = np.asarray(images)
    captions = np.asarray(captions)
    length = np.asarray(length)
    emb = np.asarray(emb)
    bf = ml_dtypes.bfloat16

    # ---- host: data-dependent index work (tiny) ----
    lens = length[:, 0]
    sort_ind = np.argsort(-lens, kind="stable").astype(np.int32)
    sorted_lens = lens[sort_ind]
    dec_len = (sorted_lens - 1).astype(lens.dtype)
    captions_s = captions[sort_ind]
    features = images[sort_ind].astype(np.float32)          # [B, E]
    embs = np.asarray(emb, np.float32)[captions_s[:, :T]]   # [B, T, E]
    X = np.concatenate([features[:, None, :], embs], axis=1)  # [B, NSTEP, E]

    def kpack(a):
        # [K, M] (K = contraction) -> [128, KC, M]: partition ki holds the
        # k = ko*128 + ki rows, contiguous per partition
        Kd, Md = a.shape
        return np.ascontiguousarray(
            a.reshape(Kd // 128, 128, Md).transpose(1, 0, 2)).astype(bf)

    bias_v = (np.asarray(b_ih, np.float32) + np.asarray(b_hh, np.float32))
    bias_pm = np.ascontiguousarray(bias_v.reshape(MT, 128).T)          # [128, MT]
    # [K, 4H] -> [128, MT, KC, 128]: partition ki, mt-major then k-chunk
    wt = np.asarray(W_ih).T.reshape(KC, 128, MT, 128)
    wihT = np.ascontiguousarray(wt.transpose(1, 2, 0, 3)).astype(bf)
    whhT = kpack(np.asarray(W_hh).T)
    fcw_f = np.zeros((E, VP), np.float32)                               # [H, Vpad]
    fcw_f[:, :V] = np.asarray(fc_w).T
    fcwT = kpack(fcw_f)
    fcb_pad = np.zeros(VT * 128, np.float32)
    fcb_pad[:V] = np.asarray(fc_b, np.float32)
    fcb_pm = np.ascontiguousarray(fcb_pad.reshape(VT, 128).T)           # [128, VT]

    t_idx = np.arange(T)
    in_maps = []
    for c in range(NCORES):
        rows = slice(c * BC, (c + 1) * BC)
        Xc = X[rows]                                        # [BC, NSTEP, E]
        xk_c = kpack(Xc.transpose(2, 1, 0).reshape(E, RX))  # [128, KC, RX]
        mask_r = (dec_len[rows][None, :] > t_idx[:, None]).reshape(R)
        mask_full = np.ascontiguousarray(
            np.broadcast_to(mask_r.astype(np.float32), (128, R)))
        in_maps.append(dict(
            xk=xk_c, wih=wihT, whh=whhT, bias=bias_pm,
            fcw=fcwT, fcb=fcb_pm, mask=mask_full,
        ))

    nc = _get_compiled()
    res = None
    for attempt in range(3):
        try:
            res = run_bass_kernel_spmd(
                nc, in_maps, list(range(NCORES)), trace=PROFILE,
            )
            break
        except Exception:
            # occasional transient NRT_EXEC_UNIT_UNRECOVERABLE on the device;
            # a clean retry has always succeeded
            if attempt == 2:
                raise
    LAST_RESULT = res

    predictions = np.empty((B, T, V), np.float32)
    for c in range(NCORES):
        pd1 = np.asarray(res.results[c]["preds1"], np.float32)  # [128, VT*R1]
        pd2 = np.asarray(res.results[c]["preds2"], np.float32)  # [128, VT*R2]
        pc = np.concatenate(
            [pd1.reshape(128, VT, R1), pd2.reshape(128, VT, R2)], axis=2)
        pc = pc.transpose(1, 0, 2).reshape(VP, R)[:V]
        predictions[c * BC:(c + 1) * BC] = (
            pc.reshape(V, T, BC).transpose(2, 1, 0))
    return predictions, captions_s, dec_len, sort_ind


# revision 23
# speedup vs baseline: 1.0406x; 1.0406x over previous
"""DecoderRNN (LSTM image-caption decoder) on 8 TRN2 NeuronCores.

Sharding: data-parallel over batch B=128 -> 16 rows per core. No collectives.
Host side does the data-dependent index work (length sort, embedding gather)
and packs operands into PE-friendly layouts; the device runs:
  1. X-proj: x_t @ W_ih^T + bias for all 20 steps as one batched matmul sweep
  2. 19 sequential LSTM-cell steps (h @ W_hh^T, gate nonlinearities)
  3. FC to vocab (10000) for all 19*16 rows, fused (psum+bias)*mask epilogue
All matmuls in bf16 with f32 PSUM accumulation.

Perf notes (trace-driven):
  - each LSTM gate group accumulates in its own PSUM bank so the scalar
    engine can start on a gate while later gates are still in matmul
  - the x-projection is injected into each gate's PSUM accumulation via an
    identity matmul, so gate pre-activations are complete in PSUM and the
    scalar engine reads them directly (no vector-engine add pass)
  - bank-clear semantics: only the first matmul of a gate group uses
    start=True (clears the whole bank's has_written bits); every other
    matmul in the group overwrites-where-unset / accumulates-where-set
"""

import numpy as np
import ml_dtypes

B, S, E, H, V = 128, 20, 512, 512, 10000
T = S - 1            # 19 decode steps
NCORES = 8
BC = B // NCORES     # 16 batch rows per core
NSTEP = S            # 20 LSTM cell evaluations (features step + 19 caption steps)
RX = NSTEP * BC      # 320 x-proj rows per core
R = T * BC           # 304 fc rows per core
KC = E // 128        # 4 contraction chunks (E == H == 512)
MT = 4 * H // 128    # 16 gate m-tiles
VT = (V + 127) // 128  # 79 vocab tiles
R1 = 8 * BC            # fc rows computed inside the recurrence (steps 0..7)
R2 = R - R1            # fc rows computed after the last step
VP = VT * 128          # vocab padded to a whole number of tiles

PROFILE = False      # set True (from test.py) to capture NTFF trace + exec time
LAST_RESULT = None   # BassKernelResults of the last run (for test.py)

_COMPILED = None


def _build():
    import concourse.mybir as mybir
    import concourse.tile as tile
    from concourse import bacc
    from concourse.masks import make_identity
    from contextlib import ExitStack

    f32 = mybir.dt.float32
    bf16 = mybir.dt.bfloat16
    AF = mybir.ActivationFunctionType
    OP = mybir.AluOpType

    nc = bacc.Bacc(None)

    # all DRAM tensors are laid out [128 partitions, ...contiguous...] so each
    # partition's payload is a single contiguous run (DMA descriptor-count,
    # not bandwidth, was the staging bottleneck with row-major layouts)
    xk = nc.declare_dram_parameter("xk", [128, KC, RX], bf16, isOutput=False)
    wih = nc.declare_dram_parameter("wih", [128, MT, KC, 128], bf16, isOutput=False)
    whh = nc.declare_dram_parameter("whh", [128, KC, 4 * H], bf16, isOutput=False)
    bias = nc.declare_dram_parameter("bias", [128, MT], f32, isOutput=False)
    fcw = nc.declare_dram_parameter("fcw", [128, KC, VP], bf16, isOutput=False)
    fcb = nc.declare_dram_parameter("fcb", [128, VT], f32, isOutput=False)
    mask = nc.declare_dram_parameter("mask", [128, R], f32, isOutput=False)
    preds1 = nc.declare_dram_parameter("preds1", [128, VT * R1], bf16, isOutput=True)
    preds2 = nc.declare_dram_parameter("preds2", [128, VT * R2], bf16, isOutput=True)

    xk_r, wih_r, whh_r, fcw_r = xk, wih, whh, fcw

    with tile.TileContext(nc) as tc, ExitStack() as ctx:
        const = ctx.enter_context(tc.tile_pool(name="const", bufs=1))
        gates = ctx.enter_context(tc.tile_pool(name="gates", bufs=2))
        fcout = ctx.enter_context(tc.tile_pool(name="fcout", bufs=3))
        # one bank per slot; x-proj + fc share the 4 rotating "big" slots,
        # the four gate groups get a dedicated bank each (4 + 4 <= 8 banks)
        ps_big = ctx.enter_context(tc.tile_pool(name="ps_big", bufs=4, space="PSUM"))
        ps_gate = ctx.enter_context(tc.tile_pool(name="ps_gate", bufs=1, space="PSUM"))

        ident = const.tile([128, 128], bf16)

        # PE warmup: dummy matmuls on an *uninitialized* scratch tile (no
        # dependencies -> issues immediately at kernel start) keep the PE busy
        # while the input DMAs stream, so the HAM clock gate reaches 2.4 GHz
        # before the first real matmul and stays there
        junk = const.tile([128, 128], bf16)
        nc.gpsimd.memset(junk[:], 0.0)
        ps_warm = ps_gate.tile([128, 4, BC], f32, tag="pg_g")
        for _ in range(160):
            nc.tensor.matmul(ps_warm[:, :, :], junk[:],
                             junk[:, :4 * BC].rearrange("p (a b) -> p a b", b=BC),
                             start=True, stop=True, skip_group_check=True)
        make_identity(nc, ident[:])

        # ---- stage weights/inputs into SBUF ----
        # one dma_start per tensor: DIRECT2D descriptor generation costs
        # ~0.6us *serially* on the issuing sequencer, so few big transfers
        # beat many small ones; fcw goes on the gpsimd sequencer so the sync
        # sequencer is free for the X-proj-critical loads
        # ---- stage weights/inputs into SBUF ----
        # many mid-size transfers spread across the 16 DMA queues; the
        # X-proj-critical loads (xk, wih) are issued first
        xk_sb = const.tile([128, KC, RX], bf16)
        for k in range(KC):
            nc.sync.dma_start(xk_sb[:, k, :], xk_r[:, k, :])
        # wih is mt-major so each quarter unlocks 4 complete X-proj groups
        wih_sb = const.tile([128, MT, KC, 128], bf16)
        for mq in range(4):
            nc.sync.dma_start(wih_sb[:, mq * 4:(mq + 1) * 4], wih_r[:, mq * 4:(mq + 1) * 4])
        bias_sb = const.tile([128, MT], f32)
        nc.sync.dma_start(bias_sb[:], bias[:])
        whh_sb = const.tile([128, KC, 4 * H], bf16)
        for k in range(KC):
            nc.sync.dma_start(whh_sb[:, k, :], whh_r[:, k, :])
        mask_sb = const.tile([128, R], f32)
        nc.sync.dma_start(mask_sb[:], mask[:])
        fcb_sb = const.tile([128, VT], f32)
        nc.sync.dma_start(fcb_sb[:], fcb[:])
        fcw_sb = const.tile([128, KC, VP], bf16)
        vtq = [0, 20 * 128, 40 * 128, 60 * 128, VP]
        for q in range(4):
            for k in range(KC):
                nc.sync.dma_start(fcw_sb[:, k, vtq[q]:vtq[q + 1]],
                                  fcw_r[:, k, vtq[q]:vtq[q + 1]])


        xp_sb = const.tile([128, MT, RX], bf16)  # x-projections + bias, all steps
        h_all = const.tile([128, KC, RX], bf16)  # h_t for every step (k-major)
        c_sb = const.tile([128, KC, BC], f32)    # cell state

        # ---- X-proj: xp[:, mt, r] = sum_k W_ihT[k, mt*128:+128].T @ x + bias ----
        for mt in range(MT):
            ps = ps_big.tile([128, RX], f32, tag="big")
            for k in range(KC):
                nc.tensor.matmul(
                    ps[:],
                    wih_sb[:, mt, k, :],
                    xk_sb[:, k, :],
                    start=(k == 0),
                    stop=(k == KC - 1),
                )
            nc.scalar.activation(
                xp_sb[:, mt, :], ps[:], AF.Identity,
                bias=bias_sb[:, mt:mt + 1], scale=1.0,
            )

        # gate m-tile bases: torch LSTMCell gate order is i, f, g, o
        I0, F0, G0, O0 = 0, 4, 8, 12

        def xp_t(g0, s):
            return xp_sb[:, g0:g0 + 4, s * BC:(s + 1) * BC]


        def fc_groups(g_list, r_lo, r_nw, preds_t, stg_tag, gsz=4, split_epi=False):
            # r_lo/r_nw: fc row window; h_all col = 16 + r
            for gi, g0 in enumerate(g_list):
                gn = min(gsz, VT - g0)
                stage = fcout.tile([128, gsz, r_nw], bf16, tag=stg_tag)
                for gj in range(gn):
                    vt = g0 + gj
                    ps = ps_big.tile([128, R], f32, tag="big")
                    for k in range(KC):
                        nc.tensor.matmul(
                            ps[:, :r_nw],
                            fcw_sb[:, k, vt * 128:(vt + 1) * 128],
                            h_all[:, k, BC + r_lo:BC + r_lo + r_nw],
                            start=(k == 0),
                            stop=(k == KC - 1),
                        )
                    if split_epi and vt % 2 == 1:
                        tmp = fcout.tile([128, r_nw], f32, tag="epi_tmp")
                        nc.scalar.activation(
                            tmp[:], ps[:, :r_nw], AF.Identity,
                            bias=fcb_sb[:, vt:vt + 1], scale=1.0,
                        )
                        nc.gpsimd.tensor_mul(
                            stage[:, gj, :], tmp[:],
                            mask_sb[:, r_lo:r_lo + r_nw],
                        )
                    else:
                        nc.vector.scalar_tensor_tensor(
                            stage[:, gj, :], ps[:, :r_nw], fcb_sb[:, vt:vt + 1],
                            mask_sb[:, r_lo:r_lo + r_nw], OP.add, OP.mult,
                        )
                eng = nc.sync if (g0 // gsz) % 2 == 0 else nc.gpsimd
                eng.dma_start(
                    preds_t[:, g0 * r_nw:(g0 + gn) * r_nw].rearrange(
                        "p (g r) -> p g r", r=r_nw),
                    stage[:, :gn, :],
                )

        # ---- step 0: h,c from features only (h_prev = 0, c_prev = 0) ----
        tg = gates.tile([128, 4, BC], f32)
        nc.scalar.activation(tg[:], xp_t(G0, 0), AF.Tanh)
        si = gates.tile([128, 4, BC], f32)
        nc.scalar.activation(si[:], xp_t(I0, 0), AF.Sigmoid)
        so = gates.tile([128, 4, BC], f32)
        nc.scalar.activation(so[:], xp_t(O0, 0), AF.Sigmoid)
        nc.vector.tensor_tensor(c_sb[:], si[:], tg[:], OP.mult)
        tc_ = gates.tile([128, 4, BC], f32)
        nc.scalar.activation(tc_[:], c_sb[:], AF.Tanh)
        nc.vector.tensor_tensor(h_all[:, :, 0:BC], so[:], tc_[:], OP.mult)

        # chunk-1 fc group schedule: 20 groups of 4 vocab tiles spread
        # over steps 9..19
        g_all = list(range(0, VT, 4))   # 20 groups of 4 vocab tiles
        chunk1_sched = {}
        gi = 0
        for s9 in range(9, NSTEP):
            take = 2 if gi + 2 <= len(g_all) else len(g_all) - gi
            if s9 == NSTEP - 1:
                take = len(g_all) - gi
            chunk1_sched[s9] = g_all[gi:gi + take]
            gi += take

        # ---- steps 1..19: full LSTM cell ----
        # gate group issue order g, f, i, o: the c-update chain (needs g, f, i)
        # runs while the o matmuls stream; the o tail is short
        for s in range(1, NSTEP):
            h_prev = h_all[:, :, (s - 1) * BC:s * BC]

            def gate_mms(g0, tag):
                ps = ps_gate.tile([128, 4, BC], f32, tag=tag)
                first = True
                for j in range(4):
                    for k in range(KC):
                        nc.tensor.matmul(
                            ps[:, j, :],
                            whh_sb[:, k, (g0 + j) * 128:(g0 + j + 1) * 128],
                            h_prev[:, k, :],
                            start=first,
                            stop=False,
                            skip_group_check=True,
                        )
                        first = False
                # inject x-proj (+bias) via identity matmul: completes the
                # gate pre-activation entirely inside PSUM
                nc.tensor.matmul(
                    ps[:, :, :], ident[:], xp_t(g0, s),
                    start=False, stop=True, skip_group_check=True,
                )
                return ps

        # g
            ps_g = gate_mms(G0, "pg_g")
            tg = gates.tile([128, 4, BC], f32)
            nc.scalar.activation(tg[:], ps_g[:, :, :], AF.Tanh)
        # f
            ps_f = gate_mms(F0, "pg_f")
            sf = gates.tile([128, 4, BC], f32)
            nc.scalar.activation(sf[:], ps_f[:, :, :], AF.Sigmoid)
            c1 = gates.tile([128, 4, BC], f32)
            nc.vector.tensor_tensor(c1[:], sf[:], c_sb[:], OP.mult)
        # i
            ps_i = gate_mms(I0, "pg_i")
            si = gates.tile([128, 4, BC], f32)
            nc.scalar.activation(si[:], ps_i[:, :, :], AF.Sigmoid)
            t2 = gates.tile([128, 4, BC], f32)
            nc.vector.tensor_tensor(t2[:], si[:], tg[:], OP.mult)
            nc.vector.tensor_tensor(c_sb[:], c1[:], t2[:], OP.add)
        # o
            ps_o = gate_mms(O0, "pg_o")
            so = gates.tile([128, 4, BC], f32)
            nc.scalar.activation(so[:], ps_o[:, :, :], AF.Sigmoid)
            tc_ = gates.tile([128, 4, BC], f32)
            nc.scalar.activation(tc_[:], c_sb[:], AF.Tanh)
            nc.vector.tensor_tensor(
                h_all[:, :, s * BC:(s + 1) * BC], so[:], tc_[:], OP.mult
            )
            # fc chunk 1 (rows for t=0..7) rides the PE/DVE gaps of the
            # gate chain once those h rows exist
            if s >= 9:
                fc_groups(chunk1_sched.get(s, []), 0, R1, preds1, "st1")

        # ---- FC tail: rows t=8..18 for every vocab tile ----
        fc_groups(list(range(0, VT, 4)), R1, R2, preds2, "st2")

    nc.compile()
    return nc


def _get_compiled():
    global _COMPILED
    if _COMPILED is None:
        _COMPILED = _build()
    return _COMPILED


def kernel(images, captions, length, emb, W_ih, W_hh, b_ih, b_hh, fc_w, fc_b):
    global LAST_RESULT
    from concourse.bass_utils import run_bass_kernel_spmd

    images = np.asarray(images)
    captions = np.asarray(captions)
    length = np.asarray(length)
    emb = np.asarray(emb)
    bf = ml_dtypes.bfloat16

    # ---- host: data-dependent index work (tiny) ----
    lens = length[:, 0]
    sort_ind = np.argsort(-lens, kind="stable").astype(np.int32)
    sorted_lens = lens[sort_ind]
    dec_len = (sorted_lens - 1).astype(lens.dtype)
    captions_s = captions[sort_ind]
    features = images[sort_ind].astype(np.float32)          # [B, E]
    embs = np.asarray(emb, np.float32)[captions_s[:, :T]]   # [B, T, E]
    X = np.concatenate([features[:, None, :], embs], axis=1)  # [B, NSTEP, E]

    def kpack(a):
        # [K, M] (K = contraction) -> [128, KC, M]: partition ki holds the
        # k = ko*128 + ki rows, contiguous per partition
        Kd, Md = a.shape
        return np.ascontiguousarray(
            a.reshape(Kd // 128, 128, Md).transpose(1, 0, 2)).astype(bf)

    bias_v = (np.asarray(b_ih, np.float32) + np.asarray(b_hh, np.float32))
    bias_pm = np.ascontiguousarray(bias_v.reshape(MT, 128).T)          # [128, MT]
    # [K, 4H] -> [128, MT, KC, 128]: partition ki, mt-major then k-chunk
    wt = np.asarray(W_ih).T.reshape(KC, 128, MT, 128)
    wihT = np.ascontiguousarray(wt.transpose(1, 2, 0, 3)).astype(bf)
    whhT = kpack(np.asarray(W_hh).T)
    fcw_f = np.zeros((E, VP), np.float32)                               # [H, Vpad]
    fcw_f[:, :V] = np.asarray(fc_w).T
    fcwT = kpack(fcw_f)
    fcb_pad = np.zeros(VT * 128, np.float32)
    fcb_pad[:V] = np.asarray(fc_b, np.float32)
    fcb_pm = np.ascontiguousarray(fcb_pad.reshape(VT, 128).T)           # [128, VT]

    t_idx = np.arange(T)
    in_maps = []
    for c in range(NCORES):
        rows = slice(c * BC, (c + 1) * BC)
        Xc = X[rows]                                        # [BC, NSTEP, E]
        xk_c = kpack(Xc.transpose(2, 1, 0).reshape(E, RX))  # [128, KC, RX]
        mask_r = (dec_len[rows][None, :] > t_idx[:, None]).reshape(R)
        mask_full = np.ascontiguousarray(
            np.broadcast_to(mask_r.astype(np.float32), (128, R)))
        in_maps.append(dict(
            xk=xk_c, wih=wihT, whh=whhT, bias=bias_pm,
            fcw=fcwT, fcb=fcb_pm, mask=mask_full,
        ))

    nc = _get_compiled()
    res = None
    for attempt in range(3):
        try:
            res = run_bass_kernel_spmd(
                nc, in_maps, list(range(NCORES)), trace=PROFILE,
            )
            break
        except Exception:
            # occasional transient NRT_EXEC_UNIT_UNRECOVERABLE on the device;
            # a clean retry has always succeeded
            if attempt == 2:
                raise
    LAST_RESULT = res

    predictions = np.empty((B, T, V), np.float32)
    for c in range(NCORES):
        pd1 = np.asarray(res.results[c]["preds1"], np.float32)  # [128, VT*R1]
        pd2 = np.asarray(res.results[c]["preds2"], np.float32)  # [128, VT*R2]
        pc = np.concatenate(
            [pd1.reshape(128, VT, R1), pd2.reshape(128, VT, R2)], axis=2)
        pc = pc.transpose(1, 0, 2).reshape(VP, R)[:V]
        predictions[c * BC:(c + 1) * BC] = (
            pc.reshape(V, T, BC).transpose(2, 1, 0))
    return predictions, captions_s, dec_len, sort_ind


# revision 24
# speedup vs baseline: 1.0556x; 1.0144x over previous
"""DecoderRNN (LSTM image-caption decoder) on 8 TRN2 NeuronCores.

Sharding: data-parallel over batch B=128 -> 16 rows per core. No collectives.
Host side does the data-dependent index work (length sort, embedding gather)
and packs operands into PE-friendly layouts; the device runs:
  1. X-proj: x_t @ W_ih^T + bias for all 20 steps as one batched matmul sweep
  2. 19 sequential LSTM-cell steps (h @ W_hh^T, gate nonlinearities)
  3. FC to vocab (10000) for all 19*16 rows, fused (psum+bias)*mask epilogue
All matmuls in bf16 with f32 PSUM accumulation.

Perf notes (trace-driven):
  - each LSTM gate group accumulates in its own PSUM bank so the scalar
    engine can start on a gate while later gates are still in matmul
  - the x-projection is injected into each gate's PSUM accumulation via an
    identity matmul, so gate pre-activations are complete in PSUM and the
    scalar engine reads them directly (no vector-engine add pass)
  - bank-clear semantics: only the first matmul of a gate group uses
    start=True (clears the whole bank's has_written bits); every other
    matmul in the group overwrites-where-unset / accumulates-where-set
"""

import numpy as np
import ml_dtypes

B, S, E, H, V = 128, 20, 512, 512, 10000
T = S - 1            # 19 decode steps
NCORES = 8
BC = B // NCORES     # 16 batch rows per core
NSTEP = S            # 20 LSTM cell evaluations (features step + 19 caption steps)
RX = NSTEP * BC      # 320 x-proj rows per core
R = T * BC           # 304 fc rows per core
KC = E // 128        # 4 contraction chunks (E == H == 512)
MT = 4 * H // 128    # 16 gate m-tiles
VT = (V + 127) // 128  # 79 vocab tiles
R1 = 8 * BC            # fc rows computed inside the recurrence (steps 0..7)
R2 = R - R1            # fc rows computed after the last step
VP = VT * 128          # vocab padded to a whole number of tiles

PROFILE = False      # set True (from test.py) to capture NTFF trace + exec time
LAST_RESULT = None   # BassKernelResults of the last run (for test.py)

_COMPILED = None


def _build():
    import concourse.mybir as mybir
    import concourse.tile as tile
    from concourse import bacc
    from concourse.masks import make_identity
    from contextlib import ExitStack

    f32 = mybir.dt.float32
    bf16 = mybir.dt.bfloat16
    AF = mybir.ActivationFunctionType
    OP = mybir.AluOpType

    nc = bacc.Bacc(None)

    # all DRAM tensors are laid out [128 partitions, ...contiguous...] so each
    # partition's payload is a single contiguous run (DMA descriptor-count,
    # not bandwidth, was the staging bottleneck with row-major layouts)
    xk = nc.declare_dram_parameter("xk", [128, KC, RX], bf16, isOutput=False)
    wih = nc.declare_dram_parameter("wih", [128, MT, KC, 128], bf16, isOutput=False)
    whh = nc.declare_dram_parameter("whh", [128, KC, 4 * H], bf16, isOutput=False)
    bias = nc.declare_dram_parameter("bias", [128, MT], f32, isOutput=False)
    fcw = nc.declare_dram_parameter("fcw", [128, KC, VP], bf16, isOutput=False)
    fcb = nc.declare_dram_parameter("fcb", [128, VT], f32, isOutput=False)
    mask = nc.declare_dram_parameter("mask", [128, R], f32, isOutput=False)
    preds1 = nc.declare_dram_parameter("preds1", [128, VT * R1], bf16, isOutput=True)
    preds2 = nc.declare_dram_parameter("preds2", [128, VT * R2], bf16, isOutput=True)

    xk_r, wih_r, whh_r, fcw_r = xk, wih, whh, fcw

    with tile.TileContext(nc) as tc, ExitStack() as ctx:
        const = ctx.enter_context(tc.tile_pool(name="const", bufs=1))
        gates = ctx.enter_context(tc.tile_pool(name="gates", bufs=2))
        fcout = ctx.enter_context(tc.tile_pool(name="fcout", bufs=3))
        # one bank per slot; x-proj + fc share the 4 rotating "big" slots,
        # the four gate groups get a dedicated bank each (4 + 4 <= 8 banks)
        ps_big = ctx.enter_context(tc.tile_pool(name="ps_big", bufs=4, space="PSUM"))
        ps_gate = ctx.enter_context(tc.tile_pool(name="ps_gate", bufs=1, space="PSUM"))

        ident = const.tile([128, 128], bf16)

        # PE warmup: dummy matmuls on an *uninitialized* scratch tile (no
        # dependencies -> issues immediately at kernel start) keep the PE busy
        # while the input DMAs stream, so the HAM clock gate reaches 2.4 GHz
        # before the first real matmul and stays there
        junk = const.tile([128, 128], bf16)
        nc.gpsimd.memset(junk[:], 0.0)
        ps_warm = ps_gate.tile([128, 4, BC], f32, tag="pg_g")
        for _ in range(160):
            nc.tensor.matmul(ps_warm[:, :, :], junk[:],
                             junk[:, :4 * BC].rearrange("p (a b) -> p a b", b=BC),
                             start=True, stop=True, skip_group_check=True)
        make_identity(nc, ident[:])

        # ---- stage weights/inputs into SBUF ----
        # one dma_start per tensor: DIRECT2D descriptor generation costs
        # ~0.6us *serially* on the issuing sequencer, so few big transfers
        # beat many small ones; fcw goes on the gpsimd sequencer so the sync
        # sequencer is free for the X-proj-critical loads
        # ---- stage weights/inputs into SBUF ----
        # many mid-size transfers spread across the 16 DMA queues; the
        # X-proj-critical loads (xk, wih) are issued first
        xk_sb = const.tile([128, KC, RX], bf16)
        for k in range(KC):
            nc.sync.dma_start(xk_sb[:, k, :], xk_r[:, k, :])
        # wih is mt-major so each pair unlocks 2 complete X-proj groups; it
        # rides the otherwise-idle gpsimd sequencer so it streams from ~1us
        # without queuing behind xk
        wih_sb = const.tile([128, MT, KC, 128], bf16)
        for mq in range(8):
            nc.gpsimd.dma_start(wih_sb[:, mq * 2:(mq + 1) * 2],
                                wih_r[:, mq * 2:(mq + 1) * 2])
        bias_sb = const.tile([128, MT], f32)
        nc.sync.dma_start(bias_sb[:], bias[:])
        whh_sb = const.tile([128, KC, 4 * H], bf16)
        for k in range(KC):
            nc.sync.dma_start(whh_sb[:, k, :], whh_r[:, k, :])
        mask_sb = const.tile([128, R], f32)
        nc.sync.dma_start(mask_sb[:], mask[:])
        fcb_sb = const.tile([128, VT], f32)
        nc.sync.dma_start(fcb_sb[:], fcb[:])
        fcw_sb = const.tile([128, KC, VP], bf16)
        vtq = [0, 20 * 128, 40 * 128, 60 * 128, VP]
        for q in range(4):
            for k in range(KC):
                nc.sync.dma_start(fcw_sb[:, k, vtq[q]:vtq[q + 1]],
                                  fcw_r[:, k, vtq[q]:vtq[q + 1]])


        xp_sb = const.tile([128, MT, RX], bf16)  # x-projections + bias, all steps
        h_all = const.tile([128, KC, RX], bf16)  # h_t for every step (k-major)
        c_sb = const.tile([128, KC, BC], f32)    # cell state

        # ---- X-proj: xp[:, mt, r] = sum_k W_ihT[k, mt*128:+128].T @ x + bias ----
        for mt in range(MT):
            ps = ps_big.tile([128, RX], f32, tag="big")
            for k in range(KC):
                nc.tensor.matmul(
                    ps[:],
                    wih_sb[:, mt, k, :],
                    xk_sb[:, k, :],
                    start=(k == 0),
                    stop=(k == KC - 1),
                )
            nc.scalar.activation(
                xp_sb[:, mt, :], ps[:], AF.Identity,
                bias=bias_sb[:, mt:mt + 1], scale=1.0,
            )

        # gate m-tile bases: torch LSTMCell gate order is i, f, g, o
        I0, F0, G0, O0 = 0, 4, 8, 12

        def xp_t(g0, s):
            return xp_sb[:, g0:g0 + 4, s * BC:(s + 1) * BC]


        def fc_groups(g_list, r_lo, r_nw, preds_t, stg_tag, gsz=4, split_epi=False):
            # r_lo/r_nw: fc row window; h_all col = 16 + r
            for gi, g0 in enumerate(g_list):
                gn = min(gsz, VT - g0)
                stage = fcout.tile([128, gsz, r_nw], bf16, tag=stg_tag)
                for gj in range(gn):
                    vt = g0 + gj
                    ps = ps_big.tile([128, R], f32, tag="big")
                    for k in range(KC):
                        nc.tensor.matmul(
                            ps[:, :r_nw],
                            fcw_sb[:, k, vt * 128:(vt + 1) * 128],
                            h_all[:, k, BC + r_lo:BC + r_lo + r_nw],
                            start=(k == 0),
                            stop=(k == KC - 1),
                        )
                    if split_epi and vt % 2 == 1:
                        tmp = fcout.tile([128, r_nw], f32, tag="epi_tmp")
                        nc.scalar.activation(
                            tmp[:], ps[:, :r_nw], AF.Identity,
                            bias=fcb_sb[:, vt:vt + 1], scale=1.0,
                        )
                        nc.gpsimd.tensor_mul(
                            stage[:, gj, :], tmp[:],
                            mask_sb[:, r_lo:r_lo + r_nw],
                        )
                    else:
                        nc.vector.scalar_tensor_tensor(
                            stage[:, gj, :], ps[:, :r_nw], fcb_sb[:, vt:vt + 1],
                            mask_sb[:, r_lo:r_lo + r_nw], OP.add, OP.mult,
                        )
                eng = nc.sync if (g0 // gsz) % 2 == 0 else nc.gpsimd
                eng.dma_start(
                    preds_t[:, g0 * r_nw:(g0 + gn) * r_nw].rearrange(
                        "p (g r) -> p g r", r=r_nw),
                    stage[:, :gn, :],
                )

        # ---- step 0: h,c from features only (h_prev = 0, c_prev = 0) ----
        tg = gates.tile([128, 4, BC], f32)
        nc.scalar.activation(tg[:], xp_t(G0, 0), AF.Tanh)
        si = gates.tile([128, 4, BC], f32)
        nc.scalar.activation(si[:], xp_t(I0, 0), AF.Sigmoid)
        so = gates.tile([128, 4, BC], f32)
        nc.scalar.activation(so[:], xp_t(O0, 0), AF.Sigmoid)
        nc.vector.tensor_tensor(c_sb[:], si[:], tg[:], OP.mult)
        tc_ = gates.tile([128, 4, BC], f32)
        nc.scalar.activation(tc_[:], c_sb[:], AF.Tanh)
        nc.vector.tensor_tensor(h_all[:, :, 0:BC], so[:], tc_[:], OP.mult)

        # chunk-1 fc group schedule: 20 groups of 4 vocab tiles spread
        # over steps 9..19
        g_all = list(range(0, VT, 4))   # 20 groups of 4 vocab tiles
        chunk1_sched = {}
        gi = 0
        for s9 in range(9, NSTEP):
            take = 2 if gi + 2 <= len(g_all) else len(g_all) - gi
            if s9 == NSTEP - 1:
                take = len(g_all) - gi
            chunk1_sched[s9] = g_all[gi:gi + take]
            gi += take

        # ---- steps 1..19: full LSTM cell ----
        # gate group issue order g, f, i, o: the c-update chain (needs g, f, i)
        # runs while the o matmuls stream; the o tail is short
        for s in range(1, NSTEP):
            h_prev = h_all[:, :, (s - 1) * BC:s * BC]

            def gate_mms(g0, tag):
                ps = ps_gate.tile([128, 4, BC], f32, tag=tag)
                first = True
                for j in range(4):
                    for k in range(KC):
                        nc.tensor.matmul(
                            ps[:, j, :],
                            whh_sb[:, k, (g0 + j) * 128:(g0 + j + 1) * 128],
                            h_prev[:, k, :],
                            start=first,
                            stop=False,
                            skip_group_check=True,
                        )
                        first = False
                # inject x-proj (+bias) via identity matmul: completes the
                # gate pre-activation entirely inside PSUM
                nc.tensor.matmul(
                    ps[:, :, :], ident[:], xp_t(g0, s),
                    start=False, stop=True, skip_group_check=True,
                )
                return ps

        # g
            ps_g = gate_mms(G0, "pg_g")
            tg = gates.tile([128, 4, BC], f32)
            nc.scalar.activation(tg[:], ps_g[:, :, :], AF.Tanh)
        # f
            ps_f = gate_mms(F0, "pg_f")
            sf = gates.tile([128, 4, BC], f32)
            nc.scalar.activation(sf[:], ps_f[:, :, :], AF.Sigmoid)
            c1 = gates.tile([128, 4, BC], f32)
            nc.vector.tensor_tensor(c1[:], sf[:], c_sb[:], OP.mult)
        # i
            ps_i = gate_mms(I0, "pg_i")
            si = gates.tile([128, 4, BC], f32)
            nc.scalar.activation(si[:], ps_i[:, :, :], AF.Sigmoid)
            t2 = gates.tile([128, 4, BC], f32)
            nc.vector.tensor_tensor(t2[:], si[:], tg[:], OP.mult)
            nc.vector.tensor_tensor(c_sb[:], c1[:], t2[:], OP.add)
        # o
            ps_o = gate_mms(O0, "pg_o")
            so = gates.tile([128, 4, BC], f32)
            nc.scalar.activation(so[:], ps_o[:, :, :], AF.Sigmoid)
            tc_ = gates.tile([128, 4, BC], f32)
            nc.scalar.activation(tc_[:], c_sb[:], AF.Tanh)
            nc.vector.tensor_tensor(
                h_all[:, :, s * BC:(s + 1) * BC], so[:], tc_[:], OP.mult
            )
            # fc chunk 1 (rows for t=0..7) rides the PE/DVE gaps of the
            # gate chain once those h rows exist
            if s >= 9:
                fc_groups(chunk1_sched.get(s, []), 0, R1, preds1, "st1")

        # ---- FC tail: rows t=8..18 for every vocab tile ----
        fc_groups(list(range(0, VT, 4)), R1, R2, preds2, "st2")

    nc.compile()
    return nc


def _get_compiled():
    global _COMPILED
    if _COMPILED is None:
        _COMPILED = _build()
    return _COMPILED


def kernel(images, captions, length, emb, W_ih, W_hh, b_ih, b_hh, fc_w, fc_b):
    global LAST_RESULT
    from concourse.bass_utils import run_bass_kernel_spmd

    images = np.asarray(images)
    captions = np.asarray(captions)
    length = np.asarray(length)
    emb = np.asarray(emb)
    bf = ml_dtypes.bfloat16

    # ---- host: data-dependent index work (tiny) ----
    lens = length[:, 0]
    sort_ind = np.argsort(-lens, kind="stable").astype(np.int32)
    sorted_lens = lens[sort_ind]
    dec_len = (sorted_lens - 1).astype(lens.dtype)
    captions_s = captions[sort_ind]
    features = images[sort_ind].astype(np.float32)          # [B, E]
    embs = np.asarray(emb, np.float32)[captions_s[:, :T]]   # [B, T, E]
    X = np.concatenate([features[:, None, :], embs], axis=1)  # [B, NSTEP, E]

    def kpack(a):
        # [K, M] (K = contraction) -> [128, KC, M]: partition ki holds the
        # k = ko*128 + ki rows, contiguous per partition
        Kd, Md = a.shape
        return np.ascontiguousarray(
            a.reshape(Kd // 128, 128, Md).transpose(1, 0, 2)).astype(bf)

    bias_v = (np.asarray(b_ih, np.float32) + np.asarray(b_hh, np.float32))
    bias_pm = np.ascontiguousarray(bias_v.reshape(MT, 128).T)          # [128, MT]
    # [K, 4H] -> [128, MT, KC, 128]: partition ki, mt-major then k-chunk
    wt = np.asarray(W_ih).T.reshape(KC, 128, MT, 128)
    wihT = np.ascontiguousarray(wt.transpose(1, 2, 0, 3)).astype(bf)
    whhT = kpack(np.asarray(W_hh).T)
    fcw_f = np.zeros((E, VP), np.float32)                               # [H, Vpad]
    fcw_f[:, :V] = np.asarray(fc_w).T
    fcwT = kpack(fcw_f)
    fcb_pad = np.zeros(VT * 128, np.float32)
    fcb_pad[:V] = np.asarray(fc_b, np.float32)
    fcb_pm = np.ascontiguousarray(fcb_pad.reshape(VT, 128).T)           # [128, VT]

    t_idx = np.arange(T)
    in_maps = []
    for c in range(NCORES):
        rows = slice(c * BC, (c + 1) * BC)
        Xc = X[rows]                                        # [BC, NSTEP, E]
        xk_c = kpack(Xc.transpose(2, 1, 0).reshape(E, RX))  # [128, KC, RX]
        mask_r = (dec_len[rows][None, :] > t_idx[:, None]).reshape(R)
        mask_full = np.ascontiguousarray(
            np.broadcast_to(mask_r.astype(np.float32), (128, R)))
        in_maps.append(dict(
            xk=xk_c, wih=wihT, whh=whhT, bias=bias_pm,
            fcw=fcwT, fcb=fcb_pm, mask=mask_full,
        ))

    nc = _get_compiled()
    res = None
    for attempt in range(3):
        try:
            res = run_bass_kernel_spmd(
                nc, in_maps, list(range(NCORES)), trace=PROFILE,
            )
            break
        except Exception:
            # occasional transient NRT_EXEC_UNIT_UNRECOVERABLE on the device;
            # a clean retry has always succeeded
            if attempt == 2:
                raise
    LAST_RESULT = res

    predictions = np.empty((B, T, V), np.float32)
    for c in range(NCORES):
        pd1 = np.asarray(res.results[c]["preds1"], np.float32)  # [128, VT*R1]
        pd2 = np.asarray(res.results[c]["preds2"], np.float32)  # [128, VT*R2]
        pc = np.concatenate(
            [pd1.reshape(128, VT, R1), pd2.reshape(128, VT, R2)], axis=2)
        pc = pc.transpose(1, 0, 2).reshape(VP, R)[:V]
        predictions[c * BC:(c + 1) * BC] = (
            pc.reshape(V, T, BC).transpose(2, 1, 0))
    return predictions, captions_s, dec_len, sort_ind


# revision 25
# speedup vs baseline: 1.0929x; 1.0353x over previous
"""DecoderRNN (LSTM image-caption decoder) on 8 TRN2 NeuronCores.

Sharding: data-parallel over batch B=128 -> 16 rows per core. No collectives.
Host side does the data-dependent index work (length sort, embedding gather)
and packs operands into PE-friendly layouts; the device runs:
  1. X-proj: x_t @ W_ih^T + bias for all 20 steps as one batched matmul sweep
  2. 19 sequential LSTM-cell steps (h @ W_hh^T, gate nonlinearities)
  3. FC to vocab (10000) for all 19*16 rows, fused (psum+bias)*mask epilogue
All matmuls in bf16 with f32 PSUM accumulation.

Perf notes (trace-driven):
  - each LSTM gate group accumulates in its own PSUM bank so the scalar
    engine can start on a gate while later gates are still in matmul
  - the x-projection is injected into each gate's PSUM accumulation via an
    identity matmul, so gate pre-activations are complete in PSUM and the
    scalar engine reads them directly (no vector-engine add pass)
  - bank-clear semantics: only the first matmul of a gate group uses
    start=True (clears the whole bank's has_written bits); every other
    matmul in the group overwrites-where-unset / accumulates-where-set
"""

import numpy as np
import ml_dtypes

B, S, E, H, V = 128, 20, 512, 512, 10000
T = S - 1            # 19 decode steps
NCORES = 8
BC = B // NCORES     # 16 batch rows per core
NSTEP = S            # 20 LSTM cell evaluations (features step + 19 caption steps)
RX = NSTEP * BC      # 320 x-proj rows per core
R = T * BC           # 304 fc rows per core
KC = E // 128        # 4 contraction chunks (E == H == 512)
MT = 4 * H // 128    # 16 gate m-tiles
VT = (V + 127) // 128  # 79 vocab tiles
R1 = 8 * BC            # fc rows computed inside the recurrence (steps 0..7)
R2 = R - R1            # fc rows computed after the last step
VP = VT * 128          # vocab padded to a whole number of tiles

PROFILE = False      # set True (from test.py) to capture NTFF trace + exec time
LAST_RESULT = None   # BassKernelResults of the last run (for test.py)

_COMPILED = None


def _build():
    import concourse.mybir as mybir
    import concourse.tile as tile
    from concourse import bacc
    from concourse.masks import make_identity
    from contextlib import ExitStack

    f32 = mybir.dt.float32
    bf16 = mybir.dt.bfloat16
    AF = mybir.ActivationFunctionType
    OP = mybir.AluOpType

    nc = bacc.Bacc(None)

    # all DRAM tensors are laid out [128 partitions, ...contiguous...] so each
    # partition's payload is a single contiguous run (DMA descriptor-count,
    # not bandwidth, was the staging bottleneck with row-major layouts)
    xk = nc.declare_dram_parameter("xk", [128, KC, RX], bf16, isOutput=False)
    wih = nc.declare_dram_parameter("wih", [128, MT, KC, 128], bf16, isOutput=False)
    whh = nc.declare_dram_parameter("whh", [128, KC, 4 * H], bf16, isOutput=False)
    bias = nc.declare_dram_parameter("bias", [128, MT], f32, isOutput=False)
    fcw = nc.declare_dram_parameter("fcw", [128, KC, VP], bf16, isOutput=False)
    fcb = nc.declare_dram_parameter("fcb", [128, VT], f32, isOutput=False)
    mask = nc.declare_dram_parameter("mask", [128, R], f32, isOutput=False)
    preds1 = nc.declare_dram_parameter("preds1", [128, VT * R1], bf16, isOutput=True)
    preds2 = nc.declare_dram_parameter("preds2", [128, VT * R2], bf16, isOutput=True)

    xk_r, wih_r, whh_r, fcw_r = xk, wih, whh, fcw

    with tile.TileContext(nc) as tc, ExitStack() as ctx:
        const = ctx.enter_context(tc.tile_pool(name="const", bufs=1))
        gates = ctx.enter_context(tc.tile_pool(name="gates", bufs=3))
        fcout = ctx.enter_context(tc.tile_pool(name="fcout", bufs=4))
        # one bank per slot; x-proj + fc share the 4 rotating "big" slots,
        # the four gate groups get a dedicated bank each (4 + 4 <= 8 banks)
        ps_big = ctx.enter_context(tc.tile_pool(name="ps_big", bufs=4, space="PSUM"))
        ps_gate = ctx.enter_context(tc.tile_pool(name="ps_gate", bufs=1, space="PSUM"))

        ident = const.tile([128, 128], bf16)

        # PE warmup: dummy matmuls on an *uninitialized* scratch tile (no
        # dependencies -> issues immediately at kernel start) keep the PE busy
        # while the input DMAs stream, so the HAM clock gate reaches 2.4 GHz
        # before the first real matmul and stays there
        junk = const.tile([128, 128], bf16)
        nc.gpsimd.memset(junk[:], 0.0)
        ps_warm = ps_gate.tile([128, 4, BC], f32, tag="pg_g")
        for _ in range(160):
            nc.tensor.matmul(ps_warm[:, :, :], junk[:],
                             junk[:, :4 * BC].rearrange("p (a b) -> p a b", b=BC),
                             start=True, stop=True, skip_group_check=True)
        make_identity(nc, ident[:])

        # ---- stage weights/inputs into SBUF ----
        # one dma_start per tensor: DIRECT2D descriptor generation costs
        # ~0.6us *serially* on the issuing sequencer, so few big transfers
        # beat many small ones; fcw goes on the gpsimd sequencer so the sync
        # sequencer is free for the X-proj-critical loads
        # ---- stage weights/inputs into SBUF ----
        # many mid-size transfers spread across the 16 DMA queues; the
        # X-proj-critical loads (xk, wih) are issued first
        xk_sb = const.tile([128, KC, RX], bf16)
        for k in range(KC):
            nc.sync.dma_start(xk_sb[:, k, :], xk_r[:, k, :])
        # wih is mt-major so each pair unlocks 2 complete X-proj groups; it
        # rides the otherwise-idle gpsimd sequencer so it streams from ~1us
        # without queuing behind xk
        wih_sb = const.tile([128, MT, KC, 128], bf16)
        for mq in range(8):
            nc.gpsimd.dma_start(wih_sb[:, mq * 2:(mq + 1) * 2],
                                wih_r[:, mq * 2:(mq + 1) * 2])
        bias_sb = const.tile([128, MT], f32)
        nc.sync.dma_start(bias_sb[:], bias[:])
        whh_sb = const.tile([128, KC, 4 * H], bf16)
        for k in range(KC):
            nc.sync.dma_start(whh_sb[:, k, :], whh_r[:, k, :])
        mask_sb = const.tile([128, R], f32)
        nc.sync.dma_start(mask_sb[:], mask[:])
        fcb_sb = const.tile([128, VT], f32)
        nc.sync.dma_start(fcb_sb[:], fcb[:])
        fcw_sb = const.tile([128, KC, VP], bf16)
        vtq = [0, 20 * 128, 40 * 128, 60 * 128, VP]
        for q in range(4):
            for k in range(KC):
                nc.sync.dma_start(fcw_sb[:, k, vtq[q]:vtq[q + 1]],
                                  fcw_r[:, k, vtq[q]:vtq[q + 1]])


        xp_sb = const.tile([128, MT, RX], bf16)  # x-projections + bias, all steps
        h_all = const.tile([128, KC, RX], bf16)  # h_t for every step (k-major)
        c_sb = const.tile([128, KC, BC], f32)    # cell state

        # ---- X-proj: xp[:, mt, r] = sum_k W_ihT[k, mt*128:+128].T @ x + bias ----
        for mt in range(MT):
            ps = ps_big.tile([128, RX], f32, tag="big")
            for k in range(KC):
                nc.tensor.matmul(
                    ps[:],
                    wih_sb[:, mt, k, :],
                    xk_sb[:, k, :],
                    start=(k == 0),
                    stop=(k == KC - 1),
                )
            nc.scalar.activation(
                xp_sb[:, mt, :], ps[:], AF.Identity,
                bias=bias_sb[:, mt:mt + 1], scale=1.0,
            )

        # gate m-tile bases: torch LSTMCell gate order is i, f, g, o
        I0, F0, G0, O0 = 0, 4, 8, 12

        def xp_t(g0, s):
            return xp_sb[:, g0:g0 + 4, s * BC:(s + 1) * BC]


        def fc_groups(g_list, r_lo, r_nw, preds_t, stg_tag, gsz=4, split_epi=False):
            # r_lo/r_nw: fc row window; h_all col = 16 + r
            for gi, g0 in enumerate(g_list):
                gn = min(gsz, VT - g0)
                stage = fcout.tile([128, gsz, r_nw], bf16, tag=stg_tag)
                for gj in range(gn):
                    vt = g0 + gj
                    ps = ps_big.tile([128, R], f32, tag="big")
                    for k in range(KC):
                        nc.tensor.matmul(
                            ps[:, :r_nw],
                            fcw_sb[:, k, vt * 128:(vt + 1) * 128],
                            h_all[:, k, BC + r_lo:BC + r_lo + r_nw],
                            start=(k == 0),
                            stop=(k == KC - 1),
                        )
                    if split_epi and vt % 2 == 1:
                        tmp = fcout.tile([128, r_nw], f32, tag="epi_tmp")
                        nc.scalar.activation(
                            tmp[:], ps[:, :r_nw], AF.Identity,
                            bias=fcb_sb[:, vt:vt + 1], scale=1.0,
                        )
                        nc.gpsimd.tensor_mul(
                            stage[:, gj, :], tmp[:],
                            mask_sb[:, r_lo:r_lo + r_nw],
                        )
                    else:
                        nc.vector.scalar_tensor_tensor(
                            stage[:, gj, :], ps[:, :r_nw], fcb_sb[:, vt:vt + 1],
                            mask_sb[:, r_lo:r_lo + r_nw], OP.add, OP.mult,
                        )
                eng = nc.sync if (g0 // gsz) % 2 == 0 else nc.gpsimd
                eng.dma_start(
                    preds_t[:, g0 * r_nw:(g0 + gn) * r_nw].rearrange(
                        "p (g r) -> p g r", r=r_nw),
                    stage[:, :gn, :],
                )

        # ---- step 0: h,c from features only (h_prev = 0, c_prev = 0) ----
        tg = gates.tile([128, 4, BC], f32)
        nc.scalar.activation(tg[:], xp_t(G0, 0), AF.Tanh)
        si = gates.tile([128, 4, BC], f32)
        nc.scalar.activation(si[:], xp_t(I0, 0), AF.Sigmoid)
        so = gates.tile([128, 4, BC], f32)
        nc.scalar.activation(so[:], xp_t(O0, 0), AF.Sigmoid)
        nc.vector.tensor_tensor(c_sb[:], si[:], tg[:], OP.mult)
        tc_ = gates.tile([128, 4, BC], f32)
        nc.scalar.activation(tc_[:], c_sb[:], AF.Tanh)
        nc.vector.tensor_tensor(h_all[:, :, 0:BC], so[:], tc_[:], OP.mult)

        # chunk-1 fc group schedule: 20 groups of 4 vocab tiles spread
        # over steps 9..19
        g_all = list(range(0, VT, 4))   # 20 groups of 4 vocab tiles
        chunk1_sched = {}
        gi = 0
        for s9 in range(9, NSTEP):
            take = 2 if gi + 2 <= len(g_all) else len(g_all) - gi
            if s9 == NSTEP - 1:
                take = len(g_all) - gi
            chunk1_sched[s9] = g_all[gi:gi + take]
            gi += take

        # ---- steps 1..19: full LSTM cell ----
        # gate group issue order g, f, i, o: the c-update chain (needs g, f, i)
        # runs while the o matmuls stream; the o tail is short
        for s in range(1, NSTEP):
            h_prev = h_all[:, :, (s - 1) * BC:s * BC]

            def gate_mms(g0, tag):
                ps = ps_gate.tile([128, 4, BC], f32, tag=tag)
                first = True
                for j in range(4):
                    for k in range(KC):
                        nc.tensor.matmul(
                            ps[:, j, :],
                            whh_sb[:, k, (g0 + j) * 128:(g0 + j + 1) * 128],
                            h_prev[:, k, :],
                            start=first,
                            stop=False,
                            skip_group_check=True,
                        )
                        first = False
                # inject x-proj (+bias) via identity matmul: completes the
                # gate pre-activation entirely inside PSUM
                nc.tensor.matmul(
                    ps[:, :, :], ident[:], xp_t(g0, s),
                    start=False, stop=True, skip_group_check=True,
                )
                return ps

        # g
            ps_g = gate_mms(G0, "pg_g")
            tg = gates.tile([128, 4, BC], f32)
            nc.scalar.activation(tg[:], ps_g[:, :, :], AF.Tanh)
        # f
            ps_f = gate_mms(F0, "pg_f")
            sf = gates.tile([128, 4, BC], f32)
            nc.scalar.activation(sf[:], ps_f[:, :, :], AF.Sigmoid)
            c1 = gates.tile([128, 4, BC], f32)
            nc.vector.tensor_tensor(c1[:], sf[:], c_sb[:], OP.mult)
        # i
            ps_i = gate_mms(I0, "pg_i")
            si = gates.tile([128, 4, BC], f32)
            nc.scalar.activation(si[:], ps_i[:, :, :], AF.Sigmoid)
            t2 = gates.tile([128, 4, BC], f32)
            nc.vector.tensor_tensor(t2[:], si[:], tg[:], OP.mult)
            nc.vector.tensor_tensor(c_sb[:], c1[:], t2[:], OP.add)
        # o
            ps_o = gate_mms(O0, "pg_o")
            so = gates.tile([128, 4, BC], f32)
            nc.scalar.activation(so[:], ps_o[:, :, :], AF.Sigmoid)
            tc_ = gates.tile([128, 4, BC], f32)
            nc.scalar.activation(tc_[:], c_sb[:], AF.Tanh)
            nc.vector.tensor_tensor(
                h_all[:, :, s * BC:(s + 1) * BC], so[:], tc_[:], OP.mult
            )
            # fc chunk 1 (rows for t=0..7) rides the PE/DVE gaps of the
            # gate chain once those h rows exist
            if s >= 9:
                fc_groups(chunk1_sched.get(s, []), 0, R1, preds1, "st1")

        # ---- FC tail: rows t=8..18 for every vocab tile ----
        fc_groups(list(range(0, VT, 4)), R1, R2, preds2, "st2")

    nc.compile()
    return nc


def _get_compiled():
    global _COMPILED
    if _COMPILED is None:
        _COMPILED = _build()
    return _COMPILED


def kernel(images, captions, length, emb, W_ih, W_hh, b_ih, b_hh, fc_w, fc_b):
    global LAST_RESULT
    from concourse.bass_utils import run_bass_kernel_spmd

    images = np.asarray(images)
    captions = np.asarray(captions)
    length = np.asarray(length)
    emb = np.asarray(emb)
    bf = ml_dtypes.bfloat16

    # ---- host: data-dependent index work (tiny) ----
    lens = length[:, 0]
    sort_ind = np.argsort(-lens, kind="stable").astype(np.int32)
    sorted_lens = lens[sort_ind]
    dec_len = (sorted_lens - 1).astype(lens.dtype)
    captions_s = captions[sort_ind]
    features = images[sort_ind].astype(np.float32)          # [B, E]
    embs = np.asarray(emb, np.float32)[captions_s[:, :T]]   # [B, T, E]
    X = np.concatenate([features[:, None, :], embs], axis=1)  # [B, NSTEP, E]

    def kpack(a):
        # [K, M] (K = contraction) -> [128, KC, M]: partition ki holds the
        # k = ko*128 + ki rows, contiguous per partition
        Kd, Md = a.shape
        return np.ascontiguousarray(
            a.reshape(Kd // 128, 128, Md).transpose(1, 0, 2)).astype(bf)

    bias_v = (np.asarray(b_ih, np.float32) + np.asarray(b_hh, np.float32))
    bias_pm = np.ascontiguousarray(bias_v.reshape(MT, 128).T)          # [128, MT]
    # [K, 4H] -> [128, MT, KC, 128]: partition ki, mt-major then k-chunk
    wt = np.asarray(W_ih).T.reshape(KC, 128, MT, 128)
    wihT = np.ascontiguousarray(wt.transpose(1, 2, 0, 3)).astype(bf)
    whhT = kpack(np.asarray(W_hh).T)
    fcw_f = np.zeros((E, VP), np.float32)                               # [H, Vpad]
    fcw_f[:, :V] = np.asarray(fc_w).T
    fcwT = kpack(fcw_f)
    fcb_pad = np.zeros(VT * 128, np.float32)
    fcb_pad[:V] = np.asarray(fc_b, np.float32)
    fcb_pm = np.ascontiguousarray(fcb_pad.reshape(VT, 128).T)           # [128, VT]

    t_idx = np.arange(T)
    in_maps = []
    for c in range(NCORES):
        rows = slice(c * BC, (c + 1) * BC)
        Xc = X[rows]                                        # [BC, NSTEP, E]
        xk_c = kpack(Xc.transpose(2, 1, 0).reshape(E, RX))  # [128, KC, RX]
        mask_r = (dec_len[rows][None, :] > t_idx[:, None]).reshape(R)
        mask_full = np.ascontiguousarray(
            np.broadcast_to(mask_r.astype(np.float32), (128, R)))
        in_maps.append(dict(
            xk=xk_c, wih=wihT, whh=whhT, bias=bias_pm,
            fcw=fcwT, fcb=fcb_pm, mask=mask_full,
        ))

    nc = _get_compiled()
    res = None
    for attempt in range(3):
        try:
            res = run_bass_kernel_spmd(
                nc, in_maps, list(range(NCORES)), trace=PROFILE,
            )
            break
        except Exception:
            # occasional transient NRT_EXEC_UNIT_UNRECOVERABLE on the device;
            # a clean retry has always succeeded
            if attempt == 2:
                raise
    LAST_RESULT = res

    predictions = np.empty((B, T, V), np.float32)
    for c in range(NCORES):
        pd1 = np.asarray(res.results[c]["preds1"], np.float32)  # [128, VT*R1]
        pd2 = np.asarray(res.results[c]["preds2"], np.float32)  # [128, VT*R2]
        pc = np.concatenate(
            [pd1.reshape(128, VT, R1), pd2.reshape(128, VT, R2)], axis=2)
        pc = pc.transpose(1, 0, 2).reshape(VP, R)[:V]
        predictions[c * BC:(c + 1) * BC] = (
            pc.reshape(V, T, BC).transpose(2, 1, 0))
    return predictions, captions_s, dec_len, sort_ind
